# revision 17
# baseline (speedup 1.0000x reference)
"""Fused LSTM cell on 8 Trainium2 NeuronCores.

Data-parallel over the batch: each core handles 1024 of the 8192 rows.
Per core, the two GEMMs (x @ Wx.T + h @ Wh.T) are fused into one
[2048]-contraction GEMM, with the gate nonlinearities + state update
fused into the PSUM eviction path.

Performance structure:
- Mixed-precision contraction: every 512-column matmul costs ~216 ns on
  the PE regardless of operand dtype, but an fp8e4 matmul in DoubleRow
  perf mode processes TWO 128-row k-chunks per instruction, halving the
  matmul count for the k-range it covers.  Gate pre-activations tolerate
  different amounts of fp8 noise (sigmoid damps, tanh doesn't; the
  o-gate only touches h), so fp8 coverage per gate is chosen as
  (i: 64, f: 32, o: 128, g: 0) of 128 (gate x tile) chunk instances,
  which lands the measured end-to-end rel err just under the 2e-2 gate
  while cutting the matmul count from 1024 to 800.
- fp8 operands are host-quantized as (A/4) and (W*4) so fp8 products
  carry the true scale and can share PSUM accumulation groups and the
  unchanged f32 bias/activation epilogue with the bf16 chunks.
- fp8 chunks are shifted away from tile 0 (o-gate only there) so tile
  0's matmul span (~24 us) matches its unavoidable activation-stream
  DMA (~24 us); the freed chunks move to mid tiles.  Tile 0 runs
  k-pair-outer so every arriving a-chunk is consumed immediately.
- A burst of dummy 128-col matmuls on memset data runs while the first
  DMAs land, holding the PE busy through the HAM cold-clock window so
  the real stream starts at 2.4 GHz.
- The SP (sync) queue carries nothing but weights after startup; bias,
  activations, c, and outputs ride the Activation queue.  Gate order
  per tile is [tanh-gate, i, f, o] with the epilogue emitted per-gate
  as its PSUM banks complete, so the post-last-matmul critical chain is
  just sigmoid(o) -> mult -> DMA.
"""

import os
import sys
import types

import ml_dtypes
import numpy as np

import concourse.bacc as bacc
import concourse.mybir as mybir
import concourse.tile as tile
from concourse.bass_utils import run_bass_kernel_spmd


def _install_ntff_shim():
    """This image's `antenv` package lacks the `axon_hooks` module, so
    `run_bass_kernel_spmd(trace=True)` would crash on import (and boot()
    skipped registering the NTFF profile hook). Recreate the module and
    register the ctypes hook so tracing works when requested; without
    this the kernel still runs fine as long as nothing asks for a trace.
    """
    try:
        import antenv.axon_hooks  # noqa: F401  (real module exists: done)

        return
    except ImportError:
        pass
    mod = types.ModuleType("antenv.axon_hooks")
    mod._hook = None
    mod.set_axon_ntff_profile_hook = lambda h: setattr(mod, "_hook", h)
    mod.get_axon_ntff_profile_hook = lambda: mod._hook
    sys.modules["antenv.axon_hooks"] = mod
    try:
        import antenv

        antenv.axon_hooks = mod
        from trn_agent_boot.trn_boot import _ntff_profile_via_ctypes

        hook = _ntff_profile_via_ctypes("/opt/axon/libaxon_pjrt.so")
        if hook is not None:
            mod._hook = hook
    except Exception:
        pass


_install_ntff_shim()

B, I, H = 8192, 1024, 1024
NCORES = 8
BL = B // NCORES        # batch rows per core
KC = (I + H) // 128     # contraction chunks of 128
NPR = KC // 2           # contraction chunk-pairs (fp8 DoubleRow unit)
HT = H // 128           # h-tiles per core
NBC = 2                 # batch chunks per h-tile
BCW = BL // NBC         # 512 columns per matmul (one PSUM bank)

F32 = mybir.dt.float32
BF16 = mybir.dt.bfloat16
E4 = mybir.dt.float8e4
DRMODE = mybir.MatmulPerfMode.DoubleRow
BF16NP = ml_dtypes.bfloat16
E4NP = ml_dtypes.float8_e4m3
AF = mybir.ActivationFunctionType
OP = mybir.AluOpType

AS_SCALE = 0.25          # host scale on fp8 activations
WS_SCALE = 4.0           # host scale on fp8 weights (product stays 1:1)

# gate completion order: tanh gate (3) first so its tanh read starts
# early; output gate (2) last so only sigmoid(o)*tanh(c) trails the
# final matmul.
GORDER = (3, 0, 1, 2)

# fp8 chunk count per (tile, gate) keyed by gate index (i, f, o, g).
# Leading chunks of each gate's k-range stay bf16; trailing chunks are
# fp8 DoubleRow pairs.  Noise cost per fp8 chunk rises i < f < o << g
# (sigmoid damping; o only touches h; g feeds c through tanh'~1), so
# the budget fills i and f first, then o.  Totals (i: 112, f: 112,
# o: 64, g: 0) of 128 simulate to rel err 1.940e-2 vs the 2e-2 gate.
# Tile 0 is all-bf16 so the startup window only needs w0 + the bf16
# activations: its 128 matmuls (27.6 us) cover that DMA, and the fp8
# stream + w1 prefetch ride the slack.
NFP8 = [
    {0: 0, 1: 0, 2: 0, 3: 0},
    {0: 16, 1: 16, 2: 10, 3: 0},
    {0: 16, 1: 16, 2: 10, 3: 0},
    {0: 16, 1: 16, 2: 10, 3: 0},
    {0: 16, 1: 16, 2: 10, 3: 0},
    {0: 16, 1: 16, 2: 8, 3: 0},
    {0: 16, 1: 16, 2: 8, 3: 0},
    {0: 16, 1: 16, 2: 8, 3: 0},
]

N_DUMMY = 30             # HAM warm-up matmuls bridging to first DMA data


def _wb_entries(t):
    """(gate, chunk) list for tile t's bf16 weights, in emission order."""
    out = []
    if t == 0:
        for p in range(NPR):
            for g in GORDER:
                if NFP8[0][g] == 0:
                    out.extend([(g, 2 * p), (g, 2 * p + 1)])
        return out
    for g in GORDER:
        for ci in range(KC - NFP8[t][g]):
            out.append((g, ci))
    return out


def _w8_entries(t):
    """(gate, pair) list for tile t's fp8 weights, in emission order."""
    out = []
    if t == 0:
        for p in range(NPR):
            for g in GORDER:
                if NFP8[0][g] == 16:
                    out.append((g, p))
        return out
    for g in GORDER:
        nb = KC - NFP8[t][g]
        for pr in range(nb // 2, NPR):
            out.append((g, pr))
    return out


WB_ENT = [_wb_entries(t) for t in range(HT)]
W8_ENT = [_w8_entries(t) for t in range(HT)]
WB_OFF = np.cumsum([0] + [len(e) for e in WB_ENT]).tolist()
W8_OFF = np.cumsum([0] + [len(e) for e in W8_ENT]).tolist()
WB_MAX = max(len(e) for e in WB_ENT[1:])    # rotating pool; t0 has its own
W8_MAX = max(len(e) for e in W8_ENT[1:])

_CACHE: dict = {}


def _build():
    nc = bacc.Bacc("TRN2", target_bir_lowering=False, debug=False)
    abT = nc.dram_tensor("ab_t", [128, KC * BL], BF16, kind="ExternalInput")
    a8T = nc.dram_tensor("a8_t", [128, KC * BL], E4, kind="ExternalInput")
    wbT = nc.dram_tensor("wb_t", [128, WB_OFF[HT] * 128], BF16,
                         kind="ExternalInput")
    w8T = nc.dram_tensor("w8_t", [128, W8_OFF[HT] * 256], E4,
                         kind="ExternalInput")
    cT = nc.dram_tensor("c_t", [H, BL], F32, kind="ExternalInput")
    bias = nc.dram_tensor("bias", [128, 4 * HT], F32, kind="ExternalInput")
    cO = nc.dram_tensor("c_out", [H, BL], F32, kind="ExternalOutput")
    hO = nc.dram_tensor("h_out", [H, BL], F32, kind="ExternalOutput")

    ab_view = abT.rearrange("p (c b) -> p c b", c=KC)
    a8_view = a8T.rearrange("p (pr two b) -> p pr two b", pr=NPR, two=2)
    wb_view = wbT.rearrange("p (n j) -> p n j", j=128)
    w8_view = w8T.rearrange("p (n two j) -> p n two j", two=2, j=128)
    c_view = cT.rearrange("(t p) b -> p t b", p=128)

    with tile.TileContext(nc) as tc:
        with (
            tc.tile_pool(name="resident", bufs=1) as res_pool,
            tc.tile_pool(name="wpool", bufs=2) as w_pool,
            tc.tile_pool(name="cpool", bufs=2) as c_pool,
            tc.tile_pool(name="opool", bufs=2) as o_pool,
            tc.tile_pool(name="act", bufs=2) as act_pool,
            tc.tile_pool(name="psum", bufs=1, space="PSUM") as psum_pool,
        ):
            ps = {}
            for g in range(4):
                for bc in range(NBC):
                    ps[g, bc] = psum_pool.tile(
                        [128, BCW], F32, tag=f"ps{g}{bc}",
                        name=f"ps{g}{bc}", bufs=1)

            # ---- HAM warm-up: dummy matmuls on memset data ----------
            dum_sb = res_pool.tile([128, 128], BF16)
            nc.vector.memset(dum_sb[:], 0.0)
            for _ in range(N_DUMMY):
                nc.tensor.matmul(ps[2, 1][:, 0:128], dum_sb[:], dum_sb[:],
                                 start=True, stop=True)

            # ---- startup choreography -------------------------------
            # SP queue:   w0 pair-groups, then w1..w7 tiles
            # Act queue:  bias | a_bf/a_f8 pairs | c01 | per-tile c/outs
            # Tile 0 runs k-pair-outer so each arriving a-chunk feeds
            # its matmuls immediately.
            bias_sb = res_pool.tile([128, 4 * HT], F32)
            ab_sb = res_pool.tile([128, KC, BL], BF16)
            a8_sb = res_pool.tile([128, NPR, 2, BL], E4)
            w0b_sb = w_pool.tile([128, len(WB_ENT[0]), 128], BF16,
                                 tag="w0b", bufs=1)
            nbw = len(WB_ENT[0]) // NPR         # bf16 entries per pair grp
            # lead each queue with exactly the first matmul's inputs:
            # sync gets w0 pair-group 0, Act gets a chunk 0 -- everything
            # else (bias, c, fp8 activations) queues behind the interleave
            for p in range(NPR):
                bsl0 = slice(p * nbw, (p + 1) * nbw)
                c0, c1 = 2 * p, 2 * p + 1
                if p == 0:
                    # halve the very first transfers so the stream can
                    # start ~1 us earlier
                    h = nbw // 2
                    nc.sync.dma_start(w0b_sb[:, 0:h], wb_view[:, 0:h])
                    nc.sync.dma_start(w0b_sb[:, h:nbw], wb_view[:, h:nbw])
                    ch = ab_view.shape[2] // 2
                    nc.scalar.dma_start(ab_sb[:, 0, 0:ch], ab_view[:, 0, 0:ch])
                    nc.scalar.dma_start(ab_sb[:, 0, ch:], ab_view[:, 0, ch:])
                else:
                    nc.sync.dma_start(w0b_sb[:, bsl0], wb_view[:, bsl0])
                    nc.scalar.dma_start(ab_sb[:, c0:c0 + 1],
                                        ab_view[:, c0:c0 + 1])
                nc.sync.dma_start(ab_sb[:, c1:c1 + 1], ab_view[:, c1:c1 + 1])
            # c for tiles 0/1 and the fp8 activations are first needed by
            # tile 0's f-epilogue / tile 1 -- stream them behind the
            # critical window on the Act queue
            nc.scalar.dma_start(bias_sb[:], bias[:])
            cp0_sb = c_pool.tile([128, 2, BL], F32, tag="cprev")
            nc.scalar.dma_start(cp0_sb[:], c_view[:, 0:2])
            for s in range(NPR // 2):
                ssl = slice(2 * s, 2 * s + 2)
                nc.scalar.dma_start(a8_sb[:, ssl], a8_view[:, ssl])

            for t in range(HT):
                if t == 0:
                    wb_sb, w8_sb = w0b_sb, None
                else:
                    wb_sb = w_pool.tile([128, WB_MAX, 128], BF16, tag="wb")
                    w8_sb = w_pool.tile([128, W8_MAX, 2, 128], E4, tag="w8")
                    nwb, nw8 = len(WB_ENT[t]), len(W8_ENT[t])
                    nc.sync.dma_start(wb_sb[:, 0:nwb],
                                      wb_view[:, WB_OFF[t]:WB_OFF[t] + nwb])
                    nc.sync.dma_start(w8_sb[:, 0:nw8],
                                      w8_view[:, W8_OFF[t]:W8_OFF[t] + nw8])

                if t == 0:
                    cp_sb = cp0_sb
                elif t % 2 == 0:
                    cp_sb = c_pool.tile([128, 2, BL], F32, tag="cprev")
                    nc.scalar.dma_start(cp_sb[:], c_view[:, t:t + 2])
                oc_sb = o_pool.tile([128, BL], F32, tag="oc")
                oh_sb = o_pool.tile([128, BL], F32, tag="oh")

                ep = {}
                for bc in range(NBC):
                    for nm in ("si", "sf", "so", "tg", "t1", "t2", "tct"):
                        ep[nm, bc] = act_pool.tile([128, BCW], F32,
                                                   tag=f"{nm}{bc}",
                                                   name=f"{nm}{bc}")

                def bias_ap(g):
                    return bias_sb[:, g * HT + t:g * HT + t + 1]

                def emit_bf(g, ci, wi, first, last):
                    for bc in range(NBC):
                        bsl = slice(bc * BCW, (bc + 1) * BCW)
                        nc.tensor.matmul(
                            ps[g, bc][:], wb_sb[:, wi, :], ab_sb[:, ci, bsl],
                            start=first, stop=last)

                def emit_dr(g, pr, wi, first, last):
                    for bc in range(NBC):
                        bsl = slice(bc * BCW, (bc + 1) * BCW)
                        nc.tensor.matmul(
                            ps[g, bc][:], w8_sb[:, wi], a8_sb[:, pr, :, bsl],
                            start=first, stop=last, perf_mode=DRMODE)

                if t == 0:
                    # k-pair-outer: all banks accumulate together so
                    # each a-chunk is consumed as soon as it arrives
                    wi_b = wi_8 = 0
                    for p in range(NPR):
                        for g in GORDER:
                            if NFP8[0][g] == 16:
                                emit_dr(g, p, wi_8, p == 0, p == NPR - 1)
                                wi_8 += 1
                            else:
                                emit_bf(g, 2 * p, wi_b, p == 0, False)
                                wi_b += 1
                                emit_bf(g, 2 * p + 1, wi_b, False,
                                        p == NPR - 1)
                                wi_b += 1

                wi_b = wi_8 = 0
                for g in GORDER:
                    nb = KC - NFP8[t][g]
                    npr = NFP8[t][g] // 2
                    if t != 0:
                        for ci in range(nb):
                            emit_bf(g, ci, wi_b, ci == 0,
                                    npr == 0 and ci == nb - 1)
                            wi_b += 1
                        for pi in range(npr):
                            emit_dr(g, nb // 2 + pi, wi_8,
                                    nb == 0 and pi == 0, pi == npr - 1)
                            wi_8 += 1
                    # emit the epilogue ops that become ready once this
                    # gate's banks stop — they overlap the next gates'
                    # matmuls and release PSUM banks early
                    for bc in range(NBC):
                        bsl = slice(bc * BCW, (bc + 1) * BCW)
                        if g == 3:
                            nc.scalar.activation(ep["tg", bc][:], ps[3, bc][:],
                                                 AF.Tanh, bias=bias_ap(3))
                        elif g == 0:
                            nc.scalar.activation(ep["si", bc][:], ps[0, bc][:],
                                                 AF.Sigmoid, bias=bias_ap(0))
                            nc.vector.tensor_tensor(
                                ep["t2", bc][:], ep["si", bc][:],
                                ep["tg", bc][:], OP.mult)
                        elif g == 1:
                            nc.scalar.activation(ep["sf", bc][:], ps[1, bc][:],
                                                 AF.Sigmoid, bias=bias_ap(1))
                            nc.vector.tensor_tensor(
                                ep["t1", bc][:], ep["sf", bc][:],
                                cp_sb[:, t % 2, bsl], OP.mult)
                            nc.vector.tensor_tensor(
                                oc_sb[:, bsl], ep["t1", bc][:],
                                ep["t2", bc][:], OP.add)
                            nc.scalar.activation(ep["tct", bc][:],
                                                 oc_sb[:, bsl], AF.Tanh)
                            # last tile: flush per-bc so only the final
                            # half-tile trails the last matmul
                            if t == HT - 1:
                                nc.scalar.dma_start(
                                    cO[t * 128:(t + 1) * 128, bsl],
                                    oc_sb[:, bsl])
                            elif bc == NBC - 1:
                                nc.scalar.dma_start(
                                    cO[t * 128:(t + 1) * 128, :], oc_sb[:])
                        else:  # g == 2
                            # The scheduler's cost model underestimates the
                            # o-gate's DoubleRow matmuls, so on the last tile
                            # it would place `so` ahead of `tct` in the
                            # scalar queue and serialize the whole tail after
                            # the final matmul; wait_until pins the o-gate
                            # epilogue last so tct runs during the o matmuls.
                            with tc.tile_wait_until(0.5, enable=(t == HT - 1)):
                                if t == HT - 1:
                                    # idle sync queue: final h_out transfers
                                    # run concurrently with the c_out ones
                                    nc.scalar.activation(ep["so", bc][:],
                                                         ps[2, bc][:],
                                                         AF.Sigmoid,
                                                         bias=bias_ap(2))
                                    nc.vector.tensor_tensor(
                                        oh_sb[:, bsl], ep["so", bc][:],
                                        ep["tct", bc][:], OP.mult)
                                    nc.sync.dma_start(
                                        hO[t * 128:(t + 1) * 128, bsl],
                                        oh_sb[:, bsl])
                                else:
                                    nc.scalar.activation(ep["so", bc][:],
                                                         ps[2, bc][:],
                                                         AF.Sigmoid,
                                                         bias=bias_ap(2))
                                    nc.vector.tensor_tensor(
                                        oh_sb[:, bsl], ep["so", bc][:],
                                        ep["tct", bc][:], OP.mult)
                                    if bc == NBC - 1:
                                        nc.scalar.dma_start(
                                            hO[t * 128:(t + 1) * 128, :],
                                            oh_sb[:])

    nc.finalize()
    return nc


def _pack_weights(W):
    """W: [4096, 2048] f32 -> (wb [128, NWB*128] bf16, w8 [128, NW8*256] e4).

    w5[g, t, j, c, p]: gate g, h-tile t, output j (128), chunk c, k p.
    lhsT layout per entry: partition = p (k within chunk), cols = j.
    """
    w5 = W.reshape(4, HT, 128, KC, 128)
    wb = np.empty((128, WB_OFF[HT], 128), dtype=BF16NP)
    w8 = np.empty((128, W8_OFF[HT], 2, 128), dtype=E4NP)
    w5b = np.ascontiguousarray(w5.transpose(0, 1, 3, 4, 2))  # g t c p j
    w5q = (w5b * WS_SCALE).astype(E4NP)
    w5bf = w5b.astype(BF16NP)
    for t in range(HT):
        for k, (g, ci) in enumerate(WB_ENT[t]):
            wb[:, WB_OFF[t] + k] = w5bf[g, t, ci]
        for k, (g, pr) in enumerate(W8_ENT[t]):
            w8[:, W8_OFF[t] + k, 0] = w5q[g, t, 2 * pr]
            w8[:, W8_OFF[t] + k, 1] = w5q[g, t, 2 * pr + 1]
    return (np.ascontiguousarray(wb).reshape(128, -1),
            np.ascontiguousarray(w8).reshape(128, -1))


def kernel(x_current, c_previous, h_previous, Wx, bx, Wh, bh):
    x = np.asarray(x_current, dtype=np.float32)
    c = np.asarray(c_previous, dtype=np.float32)
    h = np.asarray(h_previous, dtype=np.float32)
    Wx = np.asarray(Wx, dtype=np.float32)
    Wh = np.asarray(Wh, dtype=np.float32)
    bsum = np.asarray(bx, dtype=np.float32) + np.asarray(bh, dtype=np.float32)

    W = np.concatenate([Wx, Wh], axis=1)
    wb_prep, w8_prep = _pack_weights(W)
    bias_t = np.ascontiguousarray(bsum.reshape(4 * HT, 128).T)  # [128, 32]

    in_maps = []
    for core in range(NCORES):
        sl = slice(core * BL, (core + 1) * BL)
        A = np.concatenate([x[sl], h[sl]], axis=1)  # [BL, 2048]
        Ach = A.reshape(BL, KC, 128)
        ab = np.ascontiguousarray(Ach.transpose(2, 1, 0)).astype(BF16NP)
        a8 = np.ascontiguousarray(
            (A * AS_SCALE).reshape(BL, NPR, 2, 128).transpose(3, 1, 2, 0)
        ).astype(E4NP)
        in_maps.append({
            "ab_t": ab.reshape(128, -1),
            "a8_t": a8.reshape(128, -1),
            "wb_t": wb_prep,
            "w8_t": w8_prep,
            "c_t": np.ascontiguousarray(c[sl].T),
            "bias": bias_t,
        })

    if "nc" not in _CACHE:
        _CACHE["nc"] = _build()
    nc = _CACHE["nc"]

    res = run_bass_kernel_spmd(
        nc, in_maps, list(range(NCORES)),
        trace=bool(int(os.environ.get("LSTM_TRACE", "0"))),
    )
    _CACHE["last_result"] = res

    c_out = np.empty((B, H), dtype=np.float32)
    h_out = np.empty((B, H), dtype=np.float32)
    for core in range(NCORES):
        sl = slice(core * BL, (core + 1) * BL)
        c_out[sl] = res.results[core]["c_out"].T
        h_out[sl] = res.results[core]["h_out"].T
    return c_out, h_out


# revision 19
# speedup vs baseline: 1.0229x; 1.0229x over previous
"""Fused LSTM cell on 8 Trainium2 NeuronCores.

Data-parallel over the batch: each core handles 1024 of the 8192 rows.
Per core, the two GEMMs (x @ Wx.T + h @ Wh.T) are fused into one
[2048]-contraction GEMM, with the gate nonlinearities + state update
fused into the PSUM eviction path.

Performance structure:
- Mixed-precision contraction: every 512-column matmul costs ~216 ns on
  the PE regardless of operand dtype, but an fp8e4 matmul in DoubleRow
  perf mode processes TWO 128-row k-chunks per instruction, halving the
  matmul count for the k-range it covers.  Gate pre-activations tolerate
  different amounts of fp8 noise (sigmoid damps, tanh doesn't; the
  o-gate only touches h), so fp8 coverage per gate is chosen as
  (i: 64, f: 32, o: 128, g: 0) of 128 (gate x tile) chunk instances,
  which lands the measured end-to-end rel err just under the 2e-2 gate
  while cutting the matmul count from 1024 to 800.
- fp8 operands are host-quantized as (A/4) and (W*4) so fp8 products
  carry the true scale and can share PSUM accumulation groups and the
  unchanged f32 bias/activation epilogue with the bf16 chunks.
- fp8 chunks are shifted away from tile 0 (o-gate only there) so tile
  0's matmul span (~24 us) matches its unavoidable activation-stream
  DMA (~24 us); the freed chunks move to mid tiles.  Tile 0 runs
  k-pair-outer so every arriving a-chunk is consumed immediately.
- A burst of dummy 128-col matmuls on memset data runs while the first
  DMAs land, holding the PE busy through the HAM cold-clock window so
  the real stream starts at 2.4 GHz.
- The SP (sync) queue carries nothing but weights after startup; bias,
  activations, c, and outputs ride the Activation queue.  Gate order
  per tile is [tanh-gate, i, f, o] with the epilogue emitted per-gate
  as its PSUM banks complete, so the post-last-matmul critical chain is
  just sigmoid(o) -> mult -> DMA.
"""

import os
import sys
import types

import ml_dtypes
import numpy as np

import concourse.bacc as bacc
import concourse.mybir as mybir
import concourse.tile as tile
from concourse.bass_utils import run_bass_kernel_spmd


def _install_ntff_shim():
    """This image's `antenv` package lacks the `axon_hooks` module, so
    `run_bass_kernel_spmd(trace=True)` would crash on import (and boot()
    skipped registering the NTFF profile hook). Recreate the module and
    register the ctypes hook so tracing works when requested; without
    this the kernel still runs fine as long as nothing asks for a trace.
    """
    try:
        import antenv.axon_hooks  # noqa: F401  (real module exists: done)

        return
    except ImportError:
        pass
    mod = types.ModuleType("antenv.axon_hooks")
    mod._hook = None
    mod.set_axon_ntff_profile_hook = lambda h: setattr(mod, "_hook", h)
    mod.get_axon_ntff_profile_hook = lambda: mod._hook
    sys.modules["antenv.axon_hooks"] = mod
    try:
        import antenv

        antenv.axon_hooks = mod
        from trn_agent_boot.trn_boot import _ntff_profile_via_ctypes

        hook = _ntff_profile_via_ctypes("/opt/axon/libaxon_pjrt.so")
        if hook is not None:
            mod._hook = hook
    except Exception:
        pass


_install_ntff_shim()

B, I, H = 8192, 1024, 1024
NCORES = 8
BL = B // NCORES        # batch rows per core
KC = (I + H) // 128     # contraction chunks of 128
NPR = KC // 2           # contraction chunk-pairs (fp8 DoubleRow unit)
HT = H // 128           # h-tiles per core
NBC = 2                 # batch chunks per h-tile
BCW = BL // NBC         # 512 columns per matmul (one PSUM bank)

F32 = mybir.dt.float32
BF16 = mybir.dt.bfloat16
E4 = mybir.dt.float8e4
DRMODE = mybir.MatmulPerfMode.DoubleRow
BF16NP = ml_dtypes.bfloat16
E4NP = ml_dtypes.float8_e4m3
AF = mybir.ActivationFunctionType
OP = mybir.AluOpType

AS_SCALE = 0.25          # host scale on fp8 activations
WS_SCALE = 4.0           # host scale on fp8 weights (product stays 1:1)

# gate completion order: tanh gate (3) first so its tanh read starts
# early; output gate (2) last so only sigmoid(o)*tanh(c) trails the
# final matmul.
GORDER = (3, 0, 1, 2)

# fp8 chunk count per (tile, gate) keyed by gate index (i, f, o, g).
# Leading chunks of each gate's k-range stay bf16; trailing chunks are
# fp8 DoubleRow pairs.  Noise cost per fp8 chunk rises i < f < o << g
# (sigmoid damping; o only touches h; g feeds c through tanh'~1), so
# the budget fills i and f first, then o.  Totals (i: 112, f: 112,
# o: 64, g: 0) of 128 simulate to rel err 1.940e-2 vs the 2e-2 gate.
# Tile 0 is all-bf16 so the startup window only needs w0 + the bf16
# activations: its 128 matmuls (27.6 us) cover that DMA, and the fp8
# stream + w1 prefetch ride the slack.
NFP8 = [
    {0: 0, 1: 0, 2: 0, 3: 0},
    {0: 16, 1: 16, 2: 10, 3: 0},
    {0: 16, 1: 16, 2: 10, 3: 0},
    {0: 16, 1: 16, 2: 10, 3: 0},
    {0: 16, 1: 16, 2: 10, 3: 0},
    {0: 16, 1: 16, 2: 8, 3: 0},
    {0: 16, 1: 16, 2: 8, 3: 0},
    {0: 16, 1: 16, 2: 8, 3: 0},
]

N_DUMMY = 40             # HAM warm-up matmuls bridging to first DMA data


def _wb_entries(t):
    """(gate, chunk) list for tile t's bf16 weights, in emission order."""
    out = []
    if t == 0:
        for p in range(NPR):
            for g in GORDER:
                if NFP8[0][g] == 0:
                    out.extend([(g, 2 * p), (g, 2 * p + 1)])
        return out
    for g in GORDER:
        for ci in range(KC - NFP8[t][g]):
            out.append((g, ci))
    return out


def _w8_entries(t):
    """(gate, pair) list for tile t's fp8 weights, in emission order."""
    out = []
    if t == 0:
        for p in range(NPR):
            for g in GORDER:
                if NFP8[0][g] == 16:
                    out.append((g, p))
        return out
    for g in GORDER:
        nb = KC - NFP8[t][g]
        for pr in range(nb // 2, NPR):
            out.append((g, pr))
    return out


WB_ENT = [_wb_entries(t) for t in range(HT)]
W8_ENT = [_w8_entries(t) for t in range(HT)]
WB_OFF = np.cumsum([0] + [len(e) for e in WB_ENT]).tolist()
W8_OFF = np.cumsum([0] + [len(e) for e in W8_ENT]).tolist()
WB_MAX = max(len(e) for e in WB_ENT[1:])    # rotating pool; t0 has its own
W8_MAX = max(len(e) for e in W8_ENT[1:])

_CACHE: dict = {}


def _build():
    nc = bacc.Bacc("TRN2", target_bir_lowering=False, debug=False)
    abT = nc.dram_tensor("ab_t", [128, KC * BL], BF16, kind="ExternalInput")
    a8T = nc.dram_tensor("a8_t", [128, KC * BL], E4, kind="ExternalInput")
    wbT = nc.dram_tensor("wb_t", [128, WB_OFF[HT] * 128], BF16,
                         kind="ExternalInput")
    w8T = nc.dram_tensor("w8_t", [128, W8_OFF[HT] * 256], E4,
                         kind="ExternalInput")
    cT = nc.dram_tensor("c_t", [H, BL], F32, kind="ExternalInput")
    bias = nc.dram_tensor("bias", [128, 4 * HT], F32, kind="ExternalInput")
    cO = nc.dram_tensor("c_out", [H, BL], F32, kind="ExternalOutput")
    hO = nc.dram_tensor("h_out", [H, BL], F32, kind="ExternalOutput")

    ab_view = abT.rearrange("p (c b) -> p c b", c=KC)
    a8_view = a8T.rearrange("p (pr two b) -> p pr two b", pr=NPR, two=2)
    wb_view = wbT.rearrange("p (n j) -> p n j", j=128)
    w8_view = w8T.rearrange("p (n two j) -> p n two j", two=2, j=128)
    c_view = cT.rearrange("(t p) b -> p t b", p=128)

    with tile.TileContext(nc) as tc:
        with (
            tc.tile_pool(name="resident", bufs=1) as res_pool,
            tc.tile_pool(name="wpool", bufs=2) as w_pool,
            tc.tile_pool(name="cpool", bufs=2) as c_pool,
            tc.tile_pool(name="opool", bufs=2) as o_pool,
            tc.tile_pool(name="act", bufs=2) as act_pool,
            tc.tile_pool(name="psum", bufs=1, space="PSUM") as psum_pool,
        ):
            ps = {}
            for g in range(4):
                for bc in range(NBC):
                    ps[g, bc] = psum_pool.tile(
                        [128, BCW], F32, tag=f"ps{g}{bc}",
                        name=f"ps{g}{bc}", bufs=1)

            # ---- HAM warm-up: dummy matmuls on memset data ----------
            dum_sb = res_pool.tile([128, 128], BF16)
            nc.vector.memset(dum_sb[:], 0.0)
            for _ in range(N_DUMMY):
                nc.tensor.matmul(ps[2, 1][:, 0:128], dum_sb[:], dum_sb[:],
                                 start=True, stop=True)

            # ---- startup choreography -------------------------------
            # SP queue:   w0 pair-groups, then w1..w7 tiles
            # Act queue:  bias | a_bf/a_f8 pairs | c01 | per-tile c/outs
            # Tile 0 runs k-pair-outer so each arriving a-chunk feeds
            # its matmuls immediately.
            bias_sb = res_pool.tile([128, 4 * HT], F32)
            ab_sb = res_pool.tile([128, KC, BL], BF16)
            a8_sb = res_pool.tile([128, NPR, 2, BL], E4)
            w0b_sb = w_pool.tile([128, len(WB_ENT[0]), 128], BF16,
                                 tag="w0b", bufs=1)
            nbw = len(WB_ENT[0]) // NPR         # bf16 entries per pair grp
            # lead each queue with exactly the first matmul's inputs:
            # sync gets w0 pair-group 0, Act gets a chunk 0 -- everything
            # else (bias, c, fp8 activations) queues behind the interleave
            for p in range(NPR):
                bsl0 = slice(p * nbw, (p + 1) * nbw)
                c0, c1 = 2 * p, 2 * p + 1
                nc.sync.dma_start(w0b_sb[:, bsl0], wb_view[:, bsl0])
                nc.scalar.dma_start(ab_sb[:, c0:c0 + 1], ab_view[:, c0:c0 + 1])
                nc.sync.dma_start(ab_sb[:, c1:c1 + 1], ab_view[:, c1:c1 + 1])
            # c for tiles 0/1 and the fp8 activations are first needed by
            # tile 0's f-epilogue / tile 1 -- stream them behind the
            # critical window on the Act queue
            nc.scalar.dma_start(bias_sb[:], bias[:])
            cp0_sb = c_pool.tile([128, 2, BL], F32, tag="cprev")
            nc.scalar.dma_start(cp0_sb[:], c_view[:, 0:2])
            for s in range(NPR // 2):
                ssl = slice(2 * s, 2 * s + 2)
                nc.scalar.dma_start(a8_sb[:, ssl], a8_view[:, ssl])

            for t in range(HT):
                if t == 0:
                    wb_sb, w8_sb = w0b_sb, None
                else:
                    wb_sb = w_pool.tile([128, WB_MAX, 128], BF16, tag="wb")
                    w8_sb = w_pool.tile([128, W8_MAX, 2, 128], E4, tag="w8")
                    nwb, nw8 = len(WB_ENT[t]), len(W8_ENT[t])
                    nc.sync.dma_start(wb_sb[:, 0:nwb],
                                      wb_view[:, WB_OFF[t]:WB_OFF[t] + nwb])
                    nc.sync.dma_start(w8_sb[:, 0:nw8],
                                      w8_view[:, W8_OFF[t]:W8_OFF[t] + nw8])

                if t == 0:
                    cp_sb = cp0_sb
                elif t % 2 == 0:
                    cp_sb = c_pool.tile([128, 2, BL], F32, tag="cprev")
                    nc.scalar.dma_start(cp_sb[:], c_view[:, t:t + 2])
                oc_sb = o_pool.tile([128, BL], F32, tag="oc")
                oh_sb = o_pool.tile([128, BL], F32, tag="oh")

                ep = {}
                for bc in range(NBC):
                    for nm in ("si", "sf", "so", "tg", "t1", "t2", "tct"):
                        ep[nm, bc] = act_pool.tile([128, BCW], F32,
                                                   tag=f"{nm}{bc}",
                                                   name=f"{nm}{bc}")

                def bias_ap(g):
                    return bias_sb[:, g * HT + t:g * HT + t + 1]

                def emit_bf(g, ci, wi, first, last):
                    for bc in range(NBC):
                        bsl = slice(bc * BCW, (bc + 1) * BCW)
                        nc.tensor.matmul(
                            ps[g, bc][:], wb_sb[:, wi, :], ab_sb[:, ci, bsl],
                            start=first, stop=last)

                def emit_dr(g, pr, wi, first, last):
                    for bc in range(NBC):
                        bsl = slice(bc * BCW, (bc + 1) * BCW)
                        nc.tensor.matmul(
                            ps[g, bc][:], w8_sb[:, wi], a8_sb[:, pr, :, bsl],
                            start=first, stop=last, perf_mode=DRMODE)

                if t == 0:
                    # k-pair-outer: all banks accumulate together so
                    # each a-chunk is consumed as soon as it arrives
                    wi_b = wi_8 = 0
                    for p in range(NPR):
                        for g in GORDER:
                            if NFP8[0][g] == 16:
                                emit_dr(g, p, wi_8, p == 0, p == NPR - 1)
                                wi_8 += 1
                            else:
                                emit_bf(g, 2 * p, wi_b, p == 0, False)
                                wi_b += 1
                                emit_bf(g, 2 * p + 1, wi_b, False,
                                        p == NPR - 1)
                                wi_b += 1

                wi_b = wi_8 = 0
                for g in GORDER:
                    nb = KC - NFP8[t][g]
                    npr = NFP8[t][g] // 2
                    if t != 0:
                        for ci in range(nb):
                            emit_bf(g, ci, wi_b, ci == 0,
                                    npr == 0 and ci == nb - 1)
                            wi_b += 1
                        for pi in range(npr):
                            emit_dr(g, nb // 2 + pi, wi_8,
                                    nb == 0 and pi == 0, pi == npr - 1)
                            wi_8 += 1
                    # emit the epilogue ops that become ready once this
                    # gate's banks stop — they overlap the next gates'
                    # matmuls and release PSUM banks early
                    for bc in range(NBC):
                        bsl = slice(bc * BCW, (bc + 1) * BCW)
                        if g == 3:
                            nc.scalar.activation(ep["tg", bc][:], ps[3, bc][:],
                                                 AF.Tanh, bias=bias_ap(3))
                        elif g == 0:
                            nc.scalar.activation(ep["si", bc][:], ps[0, bc][:],
                                                 AF.Sigmoid, bias=bias_ap(0))
                            nc.vector.tensor_tensor(
                                ep["t2", bc][:], ep["si", bc][:],
                                ep["tg", bc][:], OP.mult)
                        elif g == 1:
                            nc.scalar.activation(ep["sf", bc][:], ps[1, bc][:],
                                                 AF.Sigmoid, bias=bias_ap(1))
                            nc.vector.tensor_tensor(
                                ep["t1", bc][:], ep["sf", bc][:],
                                cp_sb[:, t % 2, bsl], OP.mult)
                            nc.vector.tensor_tensor(
                                oc_sb[:, bsl], ep["t1", bc][:],
                                ep["t2", bc][:], OP.add)
                            nc.scalar.activation(ep["tct", bc][:],
                                                 oc_sb[:, bsl], AF.Tanh)
                            # last tile: flush per-bc so only the final
                            # half-tile trails the last matmul
                            if t == HT - 1:
                                nc.scalar.dma_start(
                                    cO[t * 128:(t + 1) * 128, bsl],
                                    oc_sb[:, bsl])
                            elif bc == NBC - 1:
                                nc.scalar.dma_start(
                                    cO[t * 128:(t + 1) * 128, :], oc_sb[:])
                        else:  # g == 2
                            # The scheduler's cost model underestimates the
                            # o-gate's DoubleRow matmuls, so on the last tile
                            # it would place `so` ahead of `tct` in the
                            # scalar queue and serialize the whole tail after
                            # the final matmul; wait_until pins the o-gate
                            # epilogue last so tct runs during the o matmuls.
                            with tc.tile_wait_until(0.5, enable=(t == HT - 1)):
                                if t == HT - 1:
                                    # idle sync queue: final h_out transfers
                                    # run concurrently with the c_out ones
                                    nc.scalar.activation(ep["so", bc][:],
                                                         ps[2, bc][:],
                                                         AF.Sigmoid,
                                                         bias=bias_ap(2))
                                    nc.vector.tensor_tensor(
                                        oh_sb[:, bsl], ep["so", bc][:],
                                        ep["tct", bc][:], OP.mult)
                                    nc.sync.dma_start(
                                        hO[t * 128:(t + 1) * 128, bsl],
                                        oh_sb[:, bsl])
                                else:
                                    nc.scalar.activation(ep["so", bc][:],
                                                         ps[2, bc][:],
                                                         AF.Sigmoid,
                                                         bias=bias_ap(2))
                                    nc.vector.tensor_tensor(
                                        oh_sb[:, bsl], ep["so", bc][:],
                                        ep["tct", bc][:], OP.mult)
                                    if bc == NBC - 1:
                                        nc.scalar.dma_start(
                                            hO[t * 128:(t + 1) * 128, :],
                                            oh_sb[:])

    nc.finalize()
    return nc


def _pack_weights(W):
    """W: [4096, 2048] f32 -> (wb [128, NWB*128] bf16, w8 [128, NW8*256] e4).

    w5[g, t, j, c, p]: gate g, h-tile t, output j (128), chunk c, k p.
    lhsT layout per entry: partition = p (k within chunk), cols = j.
    """
    w5 = W.reshape(4, HT, 128, KC, 128)
    wb = np.empty((128, WB_OFF[HT], 128), dtype=BF16NP)
    w8 = np.empty((128, W8_OFF[HT], 2, 128), dtype=E4NP)
    w5b = np.ascontiguousarray(w5.transpose(0, 1, 3, 4, 2))  # g t c p j
    w5q = (w5b * WS_SCALE).astype(E4NP)
    w5bf = w5b.astype(BF16NP)
    for t in range(HT):
        for k, (g, ci) in enumerate(WB_ENT[t]):
            wb[:, WB_OFF[t] + k] = w5bf[g, t, ci]
        for k, (g, pr) in enumerate(W8_ENT[t]):
            w8[:, W8_OFF[t] + k, 0] = w5q[g, t, 2 * pr]
            w8[:, W8_OFF[t] + k, 1] = w5q[g, t, 2 * pr + 1]
    return (np.ascontiguousarray(wb).reshape(128, -1),
            np.ascontiguousarray(w8).reshape(128, -1))


def kernel(x_current, c_previous, h_previous, Wx, bx, Wh, bh):
    x = np.asarray(x_current, dtype=np.float32)
    c = np.asarray(c_previous, dtype=np.float32)
    h = np.asarray(h_previous, dtype=np.float32)
    Wx = np.asarray(Wx, dtype=np.float32)
    Wh = np.asarray(Wh, dtype=np.float32)
    bsum = np.asarray(bx, dtype=np.float32) + np.asarray(bh, dtype=np.float32)

    W = np.concatenate([Wx, Wh], axis=1)
    wb_prep, w8_prep = _pack_weights(W)
    bias_t = np.ascontiguousarray(bsum.reshape(4 * HT, 128).T)  # [128, 32]

    in_maps = []
    for core in range(NCORES):
        sl = slice(core * BL, (core + 1) * BL)
        A = np.concatenate([x[sl], h[sl]], axis=1)  # [BL, 2048]
        Ach = A.reshape(BL, KC, 128)
        ab = np.ascontiguousarray(Ach.transpose(2, 1, 0)).astype(BF16NP)
        a8 = np.ascontiguousarray(
            (A * AS_SCALE).reshape(BL, NPR, 2, 128).transpose(3, 1, 2, 0)
        ).astype(E4NP)
        in_maps.append({
            "ab_t": ab.reshape(128, -1),
            "a8_t": a8.reshape(128, -1),
            "wb_t": wb_prep,
            "w8_t": w8_prep,
            "c_t": np.ascontiguousarray(c[sl].T),
            "bias": bias_t,
        })

    if "nc" not in _CACHE:
        _CACHE["nc"] = _build()
    nc = _CACHE["nc"]

    res = run_bass_kernel_spmd(
        nc, in_maps, list(range(NCORES)),
        trace=bool(int(os.environ.get("LSTM_TRACE", "0"))),
    )
    _CACHE["last_result"] = res

    c_out = np.empty((B, H), dtype=np.float32)
    h_out = np.empty((B, H), dtype=np.float32)
    for core in range(NCORES):
        sl = slice(core * BL, (core + 1) * BL)
        c_out[sl] = res.results[core]["c_out"].T
        h_out[sl] = res.results[core]["h_out"].T
    return c_out, h_out
